# revision 5
# baseline (speedup 1.0000x reference)
"""HGNN metapath GRU + edge-softmax message passing on 8 TRN2 NeuronCores.

Strategy (self-contained, full inputs in / full output out):
 - Edges are sharded by DESTINATION NODE RANGE: core c owns nodes
   [c*2500, (c+1)*2500) and every edge whose dst lands there (host sorts
   edges by dst).  All segment ops (softmax sum + message scatter) are then
   core-local: zero collectives.
 - The final two linear layers are folded through the segment-sum:
   out[n] = sum_h Q[n,h,:]/S[n,h] + bc,  where per-edge
   q[e,(h,i)] = exp(lrelu(a))[e,h] * (eft[e] @ BA)[.,(h,i)] is scattered
   with one-hot matmuls (one-hot matrices precomputed on host from indices).
 - Phase 1 builds a per-node table [emb(64) | pad | xn(512)] where
   xn = W_ih[n-gates] @ emb is the n-gate input contribution.  Computing it
   per NODE (20k rows) instead of per edge-slot (400k) removes the pxn psum
   groups and matmuls from the GRU inner loop entirely.
 - GRU runs feature-major (gate dims on partitions, edges on the free dim);
   the transposing dma_gather returns table rows feature-major for free.
 - Attention + one-hot scatter are interleaved per edge-tile (no big hT
   buffer, no serial tail); per-node accumulators live in SBUF f32.
"""

import sys
import numpy as np

sys.path.insert(0, "/opt/trn_rl_repo")

import ml_dtypes  # noqa: E402

N_NODES = 20000
N_CORES = 8
NPC = N_NODES // N_CORES          # 2500 nodes per core
NODE_CHUNKS = (NPC + 127) // 128  # 20
WALK = 4
FEAT = 256
HID = 64
NH = 8
HR = NH * HID                     # 512
G3 = 3 * HR                       # 1536
OUT_DIM = 16
E_TILE = 512
KC = E_TILE // 128                # 128-edge scatter chunks per tile
ROW = 640                         # table row: emb(64) pad(64) xn(512)
NP_PAD = ((N_NODES + 511) // 512) * 512  # 20480 padded node rows

bf = ml_dtypes.bfloat16


def _wrap_idx(v):
    """int array [n] -> wrapped int16 [128, n//16] layout for dma_gather."""
    n = v.shape[0]
    assert n % 16 == 0
    w = v.reshape(n // 16, 16).T.astype(np.int16)      # [16, n//16]
    return np.tile(w, (8, 1))                           # [128, n//16]


def _host_prep(x, W_mlp, b_mlp, W_ih, W_hh, b_ih, b_hh, attn, W_emb, b_emb,
               W_last, b_last, edge_metapath_indices):
    idx = np.asarray(edge_metapath_indices).astype(np.int64)
    dst = idx[:, -1]
    core = np.clip(dst // NPC, 0, N_CORES - 1)

    per_core_eids = []
    for c in range(N_CORES):
        sel = np.nonzero(core == c)[0]
        order = np.argsort(dst[sel], kind="stable")
        per_core_eids.append(sel[order])
    counts = [len(e) for e in per_core_eids]
    E_pad = max(E_TILE, ((max(counts) + E_TILE - 1) // E_TILE) * E_TILE)
    n_tiles = E_pad // E_TILE
    n_ech = E_pad // 128

    # per-core sorted/padded indices + local dst
    sidx = np.zeros((N_CORES, E_pad, WALK), np.int64)
    ldst = np.full((N_CORES, E_pad), -1000, np.int64)
    for c in range(N_CORES):
        e = per_core_eids[c]
        sidx[c, :len(e)] = idx[e]
        ldst[c, :len(e)] = dst[e] - c * NPC

    # gather indices: per tile, 4*E_TILE idxs (step-major)
    gidx = np.zeros((N_CORES, n_tiles, 128, (WALK * E_TILE) // 16), np.int16)
    for c in range(N_CORES):
        for t in range(n_tiles):
            v = sidx[c, t * E_TILE:(t + 1) * E_TILE, :].T.reshape(-1)
            gidx[c, t] = _wrap_idx(v)

    # shared scatter schedule: union over cores of node-chunks touched per
    # 128-edge chunk k
    pairs = []            # list of (k, j)
    pair_of = {}
    for k in range(n_ech):
        js = set()
        for c in range(N_CORES):
            d = ldst[c, k * 128:(k + 1) * 128]
            js |= set((d[d >= 0] // 128).tolist())
        if js:
            for j in range(min(js), max(js) + 1):
                pair_of[(k, j)] = len(pairs)
                pairs.append((k, j))
    first_k, last_k = {}, {}
    for (k, j) in pairs:
        first_k.setdefault(j, k)
        last_k[j] = k
    n_pairs = len(pairs)

    oneh = np.zeros((N_CORES, max(n_pairs, 1), 128, 128), bf)
    m_ids = np.arange(128)
    for c in range(N_CORES):
        for p, (k, j) in enumerate(pairs):
            d = ldst[c, k * 128:(k + 1) * 128]
            oneh[c, p] = (d[:, None] == (j * 128 + m_ids)[None, :]).astype(bf)

    # weights
    Wc = (np.asarray(W_last, np.float32) @ np.asarray(W_emb, np.float32))  # [16, 512]
    BA = np.zeros((HR, 136), np.float32)
    attn = np.asarray(attn, np.float32)
    for h in range(NH):
        BA[h * HID:(h + 1) * HID, h * OUT_DIM:(h + 1) * OUT_DIM] = \
            Wc[:, h * HID:(h + 1) * HID].T
        BA[h * HID:(h + 1) * HID, 128 + h] = attn[0, h, :]
    ba_p = BA.reshape(4, 128, 136).transpose(1, 0, 2).reshape(128, 4 * 136).astype(bf)

    W_hhT = np.asarray(W_hh, np.float32).T                       # [512, 1536]
    whh_p = W_hhT.reshape(4, 128, G3).transpose(1, 0, 2).reshape(128, 4 * G3).astype(bf)
    wih_p = np.asarray(W_ih, np.float32).T.astype(bf)            # [64, 1536]

    b_ih = np.asarray(b_ih, np.float32)
    b_hh = np.asarray(b_hh, np.float32)
    brz = (b_ih + b_hh)[:2 * HR].reshape(8, 128).T.copy()        # [128, 8]
    bnih = b_ih[2 * HR:].reshape(4, 128).T.copy()                # [128, 4]
    bnhh = b_hh[2 * HR:].reshape(4, 128).T.copy()                # [128, 4]
    has_bnhh = bool(np.any(bnhh != 0.0))

    b_mlp = np.asarray(b_mlp, np.float32)
    has_bmlp = bool(np.any(b_mlp != 0.0))
    bmlp_col = b_mlp.reshape(HID, 1).astype(np.float32)          # [64, 1]

    bc_vec = (np.asarray(b_emb, np.float32) @ np.asarray(W_last, np.float32).T
              + np.asarray(b_last, np.float32))                  # [16]
    bc_t = np.tile(bc_vec[None, :], (128, 1)).astype(np.float32)

    # host-pre-transposed bf16 node features [256, NP_PAD]
    x_pad = np.zeros((NP_PAD, FEAT), np.float32)
    x_pad[:N_NODES] = np.asarray(x, np.float32)
    xT = np.ascontiguousarray(x_pad.T.astype(bf))                # [256, NP_PAD]

    # W_mlp.T packed by k-chunk: [128, 2*HID] bf16
    wmlp_p = np.asarray(W_mlp, np.float32).T.astype(bf)          # [256, 64]
    wmlp_pk = wmlp_p.reshape(2, 128, HID).transpose(1, 0, 2).reshape(128, 2 * HID)

    plan = dict(E_pad=E_pad, n_tiles=n_tiles, n_ech=n_ech, pairs=pairs,
                pair_of=pair_of, first_k=first_k, last_k=last_k,
                n_pairs=n_pairs, has_bnhh=has_bnhh, has_bmlp=has_bmlp,
                flushed=set(last_k.keys()), bc_vec=bc_vec)
    shared = dict(xt=xT, wmlp=np.ascontiguousarray(wmlp_pk), wih=wih_p,
                  whh=whh_p, ba=ba_p,
                  brz=brz, bnih=bnih, bnhh=bnhh, bmlp=bmlp_col, bc=bc_t)
    percore = dict(gidx=gidx, oneh=oneh)
    return plan, shared, percore


def _build(plan):
    from contextlib import ExitStack
    import concourse.bass as bass
    import concourse.tile as tile
    from concourse import bacc, mybir

    f32 = mybir.dt.float32
    bf16 = mybir.dt.bfloat16
    i16 = mybir.dt.int16
    AF = mybir.ActivationFunctionType
    OP = mybir.AluOpType
    P = 128

    E_pad, n_tiles, n_ech = plan["E_pad"], plan["n_tiles"], plan["n_ech"]
    pairs, pair_of = plan["pairs"], plan["pair_of"]
    first_k, last_k = plan["first_k"], plan["last_k"]
    has_bnhh, has_bmlp = plan["has_bnhh"], plan["has_bmlp"]

    nc = bacc.Bacc("TRN2", target_bir_lowering=False, debug=False)

    xt_d = nc.dram_tensor("xt", [FEAT, NP_PAD], bf16, kind="ExternalInput")
    wmlp_d = nc.dram_tensor("wmlp", [P, 2 * HID], bf16, kind="ExternalInput")
    wih_d = nc.dram_tensor("wih", [HID, G3], bf16, kind="ExternalInput")
    whh_d = nc.dram_tensor("whh", [P, 4 * G3], bf16, kind="ExternalInput")
    ba_d = nc.dram_tensor("ba", [P, 4 * 136], bf16, kind="ExternalInput")
    brz_d = nc.dram_tensor("brz", [P, 8], f32, kind="ExternalInput")
    bnih_d = nc.dram_tensor("bnih", [P, 4], f32, kind="ExternalInput")
    bnhh_d = nc.dram_tensor("bnhh", [P, 4], f32, kind="ExternalInput")
    bmlp_d = nc.dram_tensor("bmlp", [HID, 1], f32, kind="ExternalInput")
    bc_d = nc.dram_tensor("bc", [P, OUT_DIM], f32, kind="ExternalInput")
    gidx_d = nc.dram_tensor("gidx", [n_tiles, P, (WALK * E_TILE) // 16], i16,
                            kind="ExternalInput")
    oneh_d = nc.dram_tensor("oneh", [max(plan["n_pairs"], 1), P, P], bf16,
                            kind="ExternalInput")
    out_d = nc.dram_tensor("out", [NODE_CHUNKS * P, OUT_DIM], f32,
                           kind="ExternalOutput")
    etab_d = nc.dram_tensor("etab", [NP_PAD, ROW], bf16, kind="Internal")

    # per-tile scatter schedule
    chunk_js = {}                       # k -> [j...]
    for (k, j) in pairs:
        chunk_js.setdefault(k, []).append(j)
    tile_js = []                        # t -> sorted js touched in tile t
    for t in range(n_tiles):
        js = set()
        for kc in range(KC):
            js |= set(chunk_js.get(t * KC + kc, []))
        tile_js.append(sorted(js))
    first_t = {j: first_k[j] // KC for j in first_k}
    last_t = {j: last_k[j] // KC for j in last_k}

    from concourse.masks import make_identity

    with tile.TileContext(nc) as tc, ExitStack() as ctx:
        wpool = ctx.enter_context(tc.tile_pool(name="w", bufs=1))
        wih_sb = wpool.tile([HID, G3], bf16, tag="wih")
        nc.sync.dma_start(wih_sb[:], wih_d[:])
        whh_sb = wpool.tile([P, 4 * G3], bf16, tag="whh")
        nc.sync.dma_start(whh_sb[:], whh_d[:])
        ba_sb = wpool.tile([P, 4 * 136], bf16, tag="ba")
        nc.sync.dma_start(ba_sb[:], ba_d[:])
        brz_sb = wpool.tile([P, 8], f32, tag="brz")
        nc.sync.dma_start(brz_sb[:], brz_d[:])
        bnih_sb = wpool.tile([P, 4], f32, tag="bnih")
        nc.sync.dma_start(bnih_sb[:], bnih_d[:])
        bnhh_sb = wpool.tile([P, 4], f32, tag="bnhh")
        nc.sync.dma_start(bnhh_sb[:], bnhh_d[:])
        bmlp_sb = wpool.tile([HID, 1], f32, tag="bmlp")
        nc.sync.dma_start(bmlp_sb[:], bmlp_d[:])
        bc_sb = wpool.tile([P, OUT_DIM], f32, tag="bc")
        nc.sync.dma_start(bc_sb[:], bc_d[:])
        wm_sb = wpool.tile([P, 2 * HID], bf16, tag="wm")
        nc.sync.dma_start(wm_sb[:], wmlp_d[:])
        ident = wpool.tile([P, P], bf16, tag="ident")
        make_identity(nc, ident[:])
        # per-node-chunk SBUF accumulators (exp-weighted features + exp sums)
        accs = wpool.tile([P, NODE_CHUNKS, 136], f32, tag="accs")
        nc.vector.memset(accs[:], 0)

        # ---------------- phase 1: node table [emb | pad | xn] ----------------
        XGRP = 512
        with tc.tile_pool(name="e_x", bufs=3) as expool, \
             tc.tile_pool(name="e_et", bufs=3) as etpool, \
             tc.tile_pool(name="e_o", bufs=6) as eopool, \
             tc.tile_pool(name="e_pe", bufs=2, space="PSUM") as epse, \
             tc.tile_pool(name="e_px", bufs=4, space="PSUM") as epsx, \
             tc.tile_pool(name="e_pt", bufs=2, space="PSUM") as epst:
            for g in range(NP_PAD // XGRP):  # 40 groups
                r0 = g * XGRP
                xa = expool.tile([P, XGRP], bf16, tag="xa")
                nc.sync.dma_start(xa[:], xt_d[0:P, r0:r0 + XGRP])
                xb = expool.tile([P, XGRP], bf16, tag="xb")
                nc.sync.dma_start(xb[:], xt_d[P:2 * P, r0:r0 + XGRP])
                # embT [64, XGRP] = W_mlp @ x.T  (feature-major)
                ept = epse.tile([HID, XGRP], f32, tag="e", space="PSUM")
                nc.tensor.matmul(ept[:], wm_sb[:, 0:HID], xa[:],
                                 start=True, stop=False)
                nc.tensor.matmul(ept[:], wm_sb[:, HID:2 * HID], xb[:],
                                 start=False, stop=True)
                embT = etpool.tile([HID, XGRP], bf16, tag="embT")
                if has_bmlp:
                    nc.vector.tensor_scalar(embT[:], ept[:], bmlp_sb[:],
                                            None, OP.add)
                else:
                    nc.scalar.copy(embT[:], ept[:])
                for sub in range(XGRP // P):
                    esb = eopool.tile([P, ROW], bf16, tag="esb")
                    # xn  [128 nodes, 512] = emb @ W_ih[n].T  (node-major)
                    pxn = epsx.tile([P, HR], f32, tag="x", space="PSUM")
                    nc.tensor.matmul(pxn[:], embT[:, sub * P:(sub + 1) * P],
                                     wih_sb[:, 2 * HR:3 * HR],
                                     start=True, stop=True)
                    nc.vector.tensor_copy(esb[:, P:ROW], pxn[:])
                    # emb node-major via PE transpose
                    ptr = epst.tile([P, HID], bf16, tag="t", space="PSUM")
                    nc.tensor.transpose(ptr[:], embT[:, sub * P:(sub + 1) * P],
                                        ident[0:HID, 0:HID])
                    nc.scalar.copy(esb[:, 0:HID], ptr[:])
                    nc.sync.dma_start(
                        etab_d[r0 + sub * P:r0 + (sub + 1) * P, :], esb[:])

        # ---------------- phase 2+3: GRU + attention + scatter ----------------
        NIDX = WALK * E_TILE
        with tc.tile_pool(name="g_idx", bufs=2) as ipool, \
             tc.tile_pool(name="g_gat", bufs=2) as gpool, \
             tc.tile_pool(name="g_rzn", bufs=2) as rznpool, \
             tc.tile_pool(name="g_h", bufs=3) as hspool, \
             tc.tile_pool(name="g_tmp", bufs=2) as tpool, \
             tc.tile_pool(name="g_pay", bufs=2) as paypool, \
             tc.tile_pool(name="g_oh", bufs=6) as ohpool, \
             tc.tile_pool(name="g_sp", bufs=2) as spool, \
             tc.tile_pool(name="g_ps", bufs=6, space="PSUM") as gpsum, \
             tc.tile_pool(name="p3_ps", bufs=2, space="PSUM") as p3psum:

            def wih_s(m):
                return wih_sb[:, m * P:(m + 1) * P]

            def whh_s(k, m):
                return whh_sb[:, k * G3 + m * P:k * G3 + (m + 1) * P]

            for t in range(n_tiles):
                idxt = ipool.tile([P, NIDX // 16], i16, tag="idx")
                nc.sync.dma_start(idxt[:], gidx_d[t])
                gat = gpool.tile([P, ROW // P, NIDX], bf16, tag="gat")
                nc.gpsimd.dma_gather(gat[:], etab_d[:], idxt[:], NIDX, NIDX,
                                     ROW, transpose=True, single_packet=False)

                def x_s(s):
                    return gat[0:HID, 0, s * E_TILE:(s + 1) * E_TILE]

                def xn_s(c, s):
                    return gat[:, 1 + c, s * E_TILE:(s + 1) * E_TILE]

                h_cur = [None] * 4
                # ---- step 0 (h = 0): r gate irrelevant unless b_hh[n] != 0
                r0_sb = [None] * 4
                if has_bnhh:
                    for c in range(4):
                        p = gpsum.tile([P, E_TILE], f32, tag="g", space="PSUM")
                        nc.tensor.matmul(p[:], wih_s(c), x_s(0), start=True, stop=True)
                        r0 = rznpool.tile([P, E_TILE], f32, tag=f"rz{c}")
                        nc.scalar.activation(r0[:], p[:], AF.Sigmoid,
                                             bias=brz_sb[:, c:c + 1])
                        r0_sb[c] = r0
                z0_sb = [None] * 4
                for c in range(4):
                    p = gpsum.tile([P, E_TILE], f32, tag="g", space="PSUM")
                    nc.tensor.matmul(p[:], wih_s(4 + c), x_s(0), start=True, stop=True)
                    z0 = rznpool.tile([P, E_TILE], bf16, tag=f"rz{4 + c}")
                    nc.scalar.activation(z0[:], p[:], AF.Sigmoid,
                                         bias=brz_sb[:, 4 + c:5 + c])
                    z0_sb[c] = z0
                for c in range(4):
                    n0 = rznpool.tile([P, E_TILE], bf16, tag=f"n{c}")
                    if has_bnhh:
                        rb = tpool.tile([P, E_TILE], f32, tag="rb")
                        nc.vector.tensor_scalar(rb[:], r0_sb[c][:],
                                                bnhh_sb[:, c:c + 1], None, OP.mult)
                        npre = tpool.tile([P, E_TILE], bf16, tag="npre")
                        nc.vector.tensor_tensor(npre[:], rb[:], xn_s(c, 0), OP.add)
                        nc.scalar.activation(n0[:], npre[:], AF.Tanh,
                                             bias=bnih_sb[:, c:c + 1])
                    else:
                        nc.scalar.activation(n0[:], xn_s(c, 0), AF.Tanh,
                                             bias=bnih_sb[:, c:c + 1])
                    zn = tpool.tile([P, E_TILE], bf16, tag="zn")
                    nc.vector.tensor_tensor(zn[:], z0_sb[c][:], n0[:], OP.mult)
                    h0 = hspool.tile([P, E_TILE], bf16, tag=f"h{c}")
                    nc.vector.tensor_tensor(h0[:], n0[:], zn[:], OP.subtract)
                    h_cur[c] = h0

                # ---- steps 1..3
                for s in range(1, WALK):
                    rz_sb = []
                    for m in range(8):
                        p = gpsum.tile([P, E_TILE], f32, tag="g", space="PSUM")
                        nc.tensor.matmul(p[:], wih_s(m), x_s(s),
                                         start=True, stop=False)
                        for k in range(4):
                            nc.tensor.matmul(p[:], whh_s(k, m), h_cur[k][:],
                                             start=False, stop=(k == 3))
                        rz = rznpool.tile([P, E_TILE], bf16, tag=f"rz{m}")
                        nc.scalar.activation(rz[:], p[:], AF.Sigmoid,
                                             bias=brz_sb[:, m:m + 1])
                        rz_sb.append(rz)
                    n_sb = []
                    for c in range(4):
                        phn = gpsum.tile([P, E_TILE], f32, tag="g", space="PSUM")
                        for k in range(4):
                            nc.tensor.matmul(phn[:], whh_s(k, 8 + c), h_cur[k][:],
                                             start=(k == 0), stop=(k == 3))
                        rhn = tpool.tile([P, E_TILE], f32, tag="rhn")
                        if has_bnhh:
                            phb = tpool.tile([P, E_TILE], f32, tag="phb")
                            nc.vector.tensor_scalar(phb[:], phn[:],
                                                    bnhh_sb[:, c:c + 1], None, OP.add)
                            nc.vector.tensor_tensor(rhn[:], rz_sb[c][:], phb[:],
                                                    OP.mult)
                        else:
                            nc.vector.tensor_tensor(rhn[:], rz_sb[c][:], phn[:],
                                                    OP.mult)
                        npre = tpool.tile([P, E_TILE], bf16, tag="npre")
                        nc.vector.tensor_tensor(npre[:], rhn[:], xn_s(c, s), OP.add)
                        nn = rznpool.tile([P, E_TILE], bf16, tag=f"n{c}")
                        nc.scalar.activation(nn[:], npre[:], AF.Tanh,
                                             bias=bnih_sb[:, c:c + 1])
                        n_sb.append(nn)
                    for c in range(4):
                        d = tpool.tile([P, E_TILE], bf16, tag="d")
                        nc.vector.tensor_tensor(d[:], h_cur[c][:], n_sb[c][:],
                                                OP.subtract)
                        zd = tpool.tile([P, E_TILE], bf16, tag="zd")
                        nc.vector.tensor_tensor(zd[:], rz_sb[4 + c][:], d[:], OP.mult)
                        hn = hspool.tile([P, E_TILE], bf16, tag=f"h{c}")
                        nc.vector.tensor_tensor(hn[:], n_sb[c][:], zd[:], OP.add)
                        h_cur[c] = hn

                # ---- attention payload per 128-edge chunk, then scatter
                pay = paypool.tile([P, KC, 136], bf16, tag="pay")
                for kc in range(KC):
                    k = t * KC + kc
                    if k not in chunk_js:
                        continue
                    pa = p3psum.tile([P, 136], f32, tag="p3", space="PSUM")
                    for kk in range(4):
                        nc.tensor.matmul(pa[:],
                                         h_cur[kk][:, kc * P:(kc + 1) * P],
                                         ba_sb[:, kk * 136:(kk + 1) * 136],
                                         start=(kk == 0), stop=(kk == 3))
                    asb = spool.tile([P, NH], f32, tag="asb")
                    nc.vector.tensor_scalar(asb[:], pa[:, 128:136], 0.01, None,
                                            OP.mult)
                    amx = spool.tile([P, NH], f32, tag="amx")
                    nc.vector.tensor_tensor(amx[:], pa[:, 128:136], asb[:], OP.max)
                    ea = spool.tile([P, NH], f32, tag="ea")
                    nc.scalar.activation(ea[:], amx[:], AF.Exp)
                    eae = spool.tile([P, NH, OUT_DIM], f32, tag="eae")
                    nc.vector.tensor_copy(
                        eae[:], ea[:, :, None].to_broadcast([P, NH, OUT_DIM]))
                    nc.vector.tensor_tensor(pay[:, kc, 0:128], pa[:, 0:128],
                                            eae[:].rearrange("p a b -> p (a b)"),
                                            OP.mult)
                    nc.scalar.copy(pay[:, kc, 128:136], ea[:])

                for j in tile_js[t]:
                    sc = p3psum.tile([P, 136], f32, tag="p3", space="PSUM")
                    ks = [kc for kc in range(KC)
                          if (t * KC + kc, j) in pair_of]
                    for i, kc in enumerate(ks):
                        pid = pair_of[(t * KC + kc, j)]
                        oh = ohpool.tile([P, P], bf16, tag="oh")
                        nc.sync.dma_start(oh[:], oneh_d[pid])
                        nc.tensor.matmul(sc[:], oh[:], pay[:, kc, :],
                                         start=(i == 0), stop=(i == len(ks) - 1),
                                         skip_group_check=True)
                    if first_t[j] == t:
                        nc.vector.tensor_copy(accs[:, j, :], sc[:])
                    else:
                        nc.vector.tensor_tensor(accs[:, j, :], accs[:, j, :],
                                                sc[:], OP.add)
                    if last_t[j] != t:
                        continue
                    # flush node-chunk j
                    scm = spool.tile([P, NH], f32, tag="scm")
                    nc.vector.tensor_scalar(scm[:], accs[:, j, 128:136], 1e-30,
                                            None, OP.max)
                    rc = spool.tile([P, NH], f32, tag="rc")
                    nc.vector.reciprocal(rc[:], scm[:])
                    rce = spool.tile([P, NH, OUT_DIM], f32, tag="rce")
                    nc.vector.tensor_copy(
                        rce[:], rc[:, :, None].to_broadcast([P, NH, OUT_DIM]))
                    wq = spool.tile([P, P], f32, tag="wq")
                    nc.vector.tensor_tensor(wq[:], accs[:, j, 0:128],
                                            rce[:].rearrange("p a b -> p (a b)"),
                                            OP.mult)
                    o16 = spool.tile([P, OUT_DIM], f32, tag="o16")
                    nc.vector.reduce_sum(
                        o16[:], wq[:].rearrange("p (h i) -> p i h", h=NH),
                        axis=mybir.AxisListType.X)
                    ob = spool.tile([P, OUT_DIM], f32, tag="ob")
                    nc.vector.tensor_tensor(ob[:], o16[:], bc_sb[:], OP.add)
                    nc.sync.dma_start(out_d[j * P:(j + 1) * P, :], ob[:])

    nc.compile()
    return nc


def kernel(**inputs):
    import os
    from concourse.bass_utils import run_bass_kernel_spmd

    num_nodes = int(inputs.pop("num_nodes", N_NODES))
    assert num_nodes == N_NODES
    plan, shared, percore = _host_prep(**inputs)
    nc = _build(plan)

    in_maps = []
    for c in range(N_CORES):
        m = dict(shared)
        m["gidx"] = np.ascontiguousarray(percore["gidx"][c])
        m["oneh"] = np.ascontiguousarray(percore["oneh"][c])
        in_maps.append(m)

    trace = bool(os.environ.get("KERNEL_TRACE"))
    res = run_bass_kernel_spmd(nc, in_maps, core_ids=list(range(N_CORES)),
                               trace=trace)
    global LAST_EXEC_NS, LAST_RESULTS
    LAST_EXEC_NS = getattr(res, "exec_time_ns", None)
    LAST_RESULTS = res

    full = np.empty((N_NODES, OUT_DIM), np.float32)
    for c in range(N_CORES):
        full[c * NPC:(c + 1) * NPC] = res.results[c]["out"][:NPC]
    # node chunks never flushed on device -> pure-bias rows
    for j in range(NODE_CHUNKS):
        if j not in plan["flushed"]:
            for c in range(N_CORES):
                lo = c * NPC + j * 128
                hi = min(c * NPC + min((j + 1) * 128, NPC), (c + 1) * NPC)
                if lo < hi:
                    full[lo:hi] = plan["bc_vec"][None, :]
    return full


# revision 10
# speedup vs baseline: 1.2894x; 1.2894x over previous
"""HGNN metapath GRU + edge-softmax message passing on 8 TRN2 NeuronCores.

Strategy (self-contained, full inputs in / full output out):
 - Edges are sharded by DESTINATION NODE RANGE: core c owns nodes
   [c*2500, (c+1)*2500) and every edge whose dst lands there (host sorts
   edges by dst).  All segment ops (softmax sum + message scatter) are then
   core-local: zero collectives.
 - The final two linear layers are folded through the segment-sum:
   out[n] = sum_h Q[n,h,:]/S[n,h] + bc,  where per-edge
   q[e,(h,i)] = exp(lrelu(a))[e,h] * (eft[e] @ BA)[.,(h,i)] is scattered
   with one-hot matmuls (one-hot matrices precomputed on host from indices).
 - Phase 1 consumes host-pre-transposed bf16 features (xT) so the node
   embedding table is built with N=512 feature-major matmuls + cheap PE
   transposes -- no fp32 transposes or casts.
 - GRU runs feature-major (gate dims on partitions, edges on the free dim);
   the transposing dma_gather returns table rows feature-major for free.
 - Attention + one-hot scatter are interleaved per edge-tile (no big hT
   buffer, no serial tail); per-node accumulators live in SBUF f32.
"""

import sys
import numpy as np

sys.path.insert(0, "/opt/trn_rl_repo")

import ml_dtypes  # noqa: E402

N_NODES = 20000
N_CORES = 8
NPC = N_NODES // N_CORES          # 2500 nodes per core
NODE_CHUNKS = (NPC + 127) // 128  # 20
WALK = 4
FEAT = 256
HID = 64
NH = 8
HR = NH * HID                     # 512
G3 = 3 * HR                       # 1536
OUT_DIM = 16
E_TILE = 512
KC = E_TILE // 128                # 128-edge scatter chunks per tile
ROW = 128                         # table row: emb(64) pad(64)
NP_PAD = ((N_NODES + 511) // 512) * 512  # 20480 padded node rows

bf = ml_dtypes.bfloat16


def _wrap_idx(v):
    """int array [n] -> wrapped int16 [128, n//16] layout for dma_gather."""
    n = v.shape[0]
    assert n % 16 == 0
    w = v.reshape(n // 16, 16).T.astype(np.int16)      # [16, n//16]
    return np.tile(w, (8, 1))                           # [128, n//16]


def _host_prep(x, W_mlp, b_mlp, W_ih, W_hh, b_ih, b_hh, attn, W_emb, b_emb,
               W_last, b_last, edge_metapath_indices):
    idx = np.asarray(edge_metapath_indices).astype(np.int64)
    dst = idx[:, -1]
    core = np.clip(dst // NPC, 0, N_CORES - 1)

    per_core_eids = []
    for c in range(N_CORES):
        sel = np.nonzero(core == c)[0]
        order = np.argsort(dst[sel], kind="stable")
        per_core_eids.append(sel[order])
    counts = [len(e) for e in per_core_eids]
    E_pad = max(E_TILE, ((max(counts) + E_TILE - 1) // E_TILE) * E_TILE)
    n_tiles = E_pad // E_TILE
    n_ech = E_pad // 128

    # per-core sorted/padded indices + local dst
    sidx = np.zeros((N_CORES, E_pad, WALK), np.int64)
    ldst = np.full((N_CORES, E_pad), -1000, np.int64)
    for c in range(N_CORES):
        e = per_core_eids[c]
        sidx[c, :len(e)] = idx[e]
        ldst[c, :len(e)] = dst[e] - c * NPC

    # gather indices: per tile, 4*E_TILE idxs (step-major)
    gidx = np.zeros((N_CORES, n_tiles, 128, (WALK * E_TILE) // 16), np.int16)
    for c in range(N_CORES):
        for t in range(n_tiles):
            v = sidx[c, t * E_TILE:(t + 1) * E_TILE, :].T.reshape(-1)
            gidx[c, t] = _wrap_idx(v)

    # shared scatter schedule: union over cores of node-chunks touched per
    # 128-edge chunk k
    pairs = []            # list of (k, j)
    pair_of = {}
    for k in range(n_ech):
        js = set()
        for c in range(N_CORES):
            d = ldst[c, k * 128:(k + 1) * 128]
            js |= set((d[d >= 0] // 128).tolist())
        if js:
            for j in range(min(js), max(js) + 1):
                pair_of[(k, j)] = len(pairs)
                pairs.append((k, j))
    first_k, last_k = {}, {}
    for (k, j) in pairs:
        first_k.setdefault(j, k)
        last_k[j] = k
    n_pairs = len(pairs)

    oneh = np.zeros((N_CORES, max(n_pairs, 1), 128, 128), bf)
    m_ids = np.arange(128)
    for c in range(N_CORES):
        for p, (k, j) in enumerate(pairs):
            d = ldst[c, k * 128:(k + 1) * 128]
            oneh[c, p] = (d[:, None] == (j * 128 + m_ids)[None, :]).astype(bf)

    # weights
    Wc = (np.asarray(W_last, np.float32) @ np.asarray(W_emb, np.float32))  # [16, 512]
    BA = np.zeros((HR, 136), np.float32)
    attn = np.asarray(attn, np.float32)
    for h in range(NH):
        BA[h * HID:(h + 1) * HID, h * OUT_DIM:(h + 1) * OUT_DIM] = \
            Wc[:, h * HID:(h + 1) * HID].T
        BA[h * HID:(h + 1) * HID, 128 + h] = attn[0, h, :]
    ba_p = BA.reshape(4, 128, 136).transpose(1, 0, 2).reshape(128, 4 * 136).astype(bf)

    W_hhT = np.asarray(W_hh, np.float32).T                       # [512, 1536]
    whh_p = W_hhT.reshape(4, 128, G3).transpose(1, 0, 2).reshape(128, 4 * G3).astype(bf)
    wih_p = np.asarray(W_ih, np.float32).T.astype(bf)            # [64, 1536]

    b_ih = np.asarray(b_ih, np.float32)
    b_hh = np.asarray(b_hh, np.float32)
    brz = (b_ih + b_hh)[:2 * HR].reshape(8, 128).T.copy()        # [128, 8]
    bnih = b_ih[2 * HR:].reshape(4, 128).T.copy()                # [128, 4]
    bnhh = b_hh[2 * HR:].reshape(4, 128).T.copy()                # [128, 4]
    has_bnhh = bool(np.any(bnhh != 0.0))

    b_mlp = np.asarray(b_mlp, np.float32)
    has_bmlp = bool(np.any(b_mlp != 0.0))
    bmlp_col = b_mlp.reshape(HID, 1).astype(np.float32)          # [64, 1]

    bc_vec = (np.asarray(b_emb, np.float32) @ np.asarray(W_last, np.float32).T
              + np.asarray(b_last, np.float32))                  # [16]
    bc_t = np.tile(bc_vec[None, :], (128, 1)).astype(np.float32)

    # host-pre-transposed bf16 node features [256, NP_PAD]
    x_pad = np.zeros((NP_PAD, FEAT), np.float32)
    x_pad[:N_NODES] = np.asarray(x, np.float32)
    xT = np.ascontiguousarray(x_pad.T.astype(bf))                # [256, NP_PAD]

    # W_mlp.T packed by k-chunk: [128, 2*HID] bf16
    wmlp_p = np.asarray(W_mlp, np.float32).T.astype(bf)          # [256, 64]
    wmlp_pk = wmlp_p.reshape(2, 128, HID).transpose(1, 0, 2).reshape(128, 2 * HID)

    plan = dict(E_pad=E_pad, n_tiles=n_tiles, n_ech=n_ech, pairs=pairs,
                pair_of=pair_of, first_k=first_k, last_k=last_k,
                n_pairs=n_pairs, has_bnhh=has_bnhh, has_bmlp=has_bmlp,
                flushed=set(last_k.keys()), bc_vec=bc_vec)
    shared = dict(xt=xT, wmlp=np.ascontiguousarray(wmlp_pk), wih=wih_p,
                  whh=whh_p, ba=ba_p,
                  brz=brz, bnih=bnih, bnhh=bnhh, bmlp=bmlp_col, bc=bc_t)
    percore = dict(gidx=gidx, oneh=oneh)
    return plan, shared, percore


def _build(plan):
    from contextlib import ExitStack
    import concourse.bass as bass
    import concourse.tile as tile
    from concourse import bacc, mybir

    f32 = mybir.dt.float32
    bf16 = mybir.dt.bfloat16
    i16 = mybir.dt.int16
    AF = mybir.ActivationFunctionType
    OP = mybir.AluOpType
    P = 128

    E_pad, n_tiles, n_ech = plan["E_pad"], plan["n_tiles"], plan["n_ech"]
    pairs, pair_of = plan["pairs"], plan["pair_of"]
    first_k, last_k = plan["first_k"], plan["last_k"]
    has_bnhh, has_bmlp = plan["has_bnhh"], plan["has_bmlp"]

    nc = bacc.Bacc("TRN2", target_bir_lowering=False, debug=False)

    xt_d = nc.dram_tensor("xt", [FEAT, NP_PAD], bf16, kind="ExternalInput")
    wmlp_d = nc.dram_tensor("wmlp", [P, 2 * HID], bf16, kind="ExternalInput")
    wih_d = nc.dram_tensor("wih", [HID, G3], bf16, kind="ExternalInput")
    whh_d = nc.dram_tensor("whh", [P, 4 * G3], bf16, kind="ExternalInput")
    ba_d = nc.dram_tensor("ba", [P, 4 * 136], bf16, kind="ExternalInput")
    brz_d = nc.dram_tensor("brz", [P, 8], f32, kind="ExternalInput")
    bnih_d = nc.dram_tensor("bnih", [P, 4], f32, kind="ExternalInput")
    bnhh_d = nc.dram_tensor("bnhh", [P, 4], f32, kind="ExternalInput")
    bmlp_d = nc.dram_tensor("bmlp", [HID, 1], f32, kind="ExternalInput")
    bc_d = nc.dram_tensor("bc", [P, OUT_DIM], f32, kind="ExternalInput")
    gidx_d = nc.dram_tensor("gidx", [n_tiles, P, (WALK * E_TILE) // 16], i16,
                            kind="ExternalInput")
    oneh_d = nc.dram_tensor("oneh", [max(plan["n_pairs"], 1), P, P], bf16,
                            kind="ExternalInput")
    out_d = nc.dram_tensor("out", [NODE_CHUNKS * P, OUT_DIM], f32,
                           kind="ExternalOutput")
    etab_d = nc.dram_tensor("etab", [NP_PAD, ROW], bf16, kind="Internal")

    # per-tile scatter schedule
    chunk_js = {}                       # k -> [j...]
    for (k, j) in pairs:
        chunk_js.setdefault(k, []).append(j)
    tile_js = []                        # t -> sorted js touched in tile t
    for t in range(n_tiles):
        js = set()
        for kc in range(KC):
            js |= set(chunk_js.get(t * KC + kc, []))
        tile_js.append(sorted(js))
    first_t = {j: first_k[j] // KC for j in first_k}
    last_t = {j: last_k[j] // KC for j in last_k}

    from concourse.masks import make_identity

    with tile.TileContext(nc) as tc, ExitStack() as ctx:
        wpool = ctx.enter_context(tc.tile_pool(name="w", bufs=1))
        wih_sb = wpool.tile([HID, G3], bf16, tag="wih")
        nc.sync.dma_start(wih_sb[:], wih_d[:])
        whh_sb = wpool.tile([P, 4 * G3], bf16, tag="whh")
        nc.sync.dma_start(whh_sb[:], whh_d[:])
        ba_sb = wpool.tile([P, 4 * 136], bf16, tag="ba")
        nc.sync.dma_start(ba_sb[:], ba_d[:])
        brz_sb = wpool.tile([P, 8], f32, tag="brz")
        nc.sync.dma_start(brz_sb[:], brz_d[:])
        bnih_sb = wpool.tile([P, 4], f32, tag="bnih")
        nc.sync.dma_start(bnih_sb[:], bnih_d[:])
        bnhh_sb = wpool.tile([P, 4], f32, tag="bnhh")
        nc.sync.dma_start(bnhh_sb[:], bnhh_d[:])
        bmlp_sb = wpool.tile([HID, 1], f32, tag="bmlp")
        nc.sync.dma_start(bmlp_sb[:], bmlp_d[:])
        bc_sb = wpool.tile([P, OUT_DIM], f32, tag="bc")
        nc.sync.dma_start(bc_sb[:], bc_d[:])
        wm_sb = wpool.tile([P, 2 * HID], bf16, tag="wm")
        nc.sync.dma_start(wm_sb[:], wmlp_d[:])
        ident = wpool.tile([P, P], bf16, tag="ident")
        make_identity(nc, ident[:])
        # per-node-chunk SBUF accumulators (exp-weighted features + exp sums)
        accs = wpool.tile([P, NODE_CHUNKS, 136], f32, tag="accs")
        nc.vector.memset(accs[:], 0)

        # ---------------- phase 1: node embedding table ----------------
        # etab columns 64:128 hold garbage (never read: gather consumers
        # slice [0:HID]).
        XGRP = 512
        with tc.tile_pool(name="e_x", bufs=3) as expool, \
             tc.tile_pool(name="e_et", bufs=3) as etpool, \
             tc.tile_pool(name="e_o", bufs=6) as eopool, \
             tc.tile_pool(name="e_pe", bufs=2, space="PSUM") as epse, \
             tc.tile_pool(name="e_pt", bufs=4, space="PSUM") as epst:
            for g in range(NP_PAD // XGRP):  # 40 groups
                r0 = g * XGRP
                xa = expool.tile([P, XGRP], bf16, tag="xa")
                nc.sync.dma_start(xa[:], xt_d[0:P, r0:r0 + XGRP])
                xb = expool.tile([P, XGRP], bf16, tag="xb")
                nc.sync.dma_start(xb[:], xt_d[P:2 * P, r0:r0 + XGRP])
                # embT [64, XGRP] = W_mlp @ x.T  (feature-major)
                ept = epse.tile([HID, XGRP], f32, tag="e", space="PSUM")
                nc.tensor.matmul(ept[:], wm_sb[:, 0:HID], xa[:],
                                 start=True, stop=False)
                nc.tensor.matmul(ept[:], wm_sb[:, HID:2 * HID], xb[:],
                                 start=False, stop=True)
                embT = etpool.tile([HID, XGRP], bf16, tag="embT")
                if has_bmlp:
                    nc.vector.tensor_scalar(embT[:], ept[:], bmlp_sb[:],
                                            None, OP.add)
                else:
                    nc.scalar.copy(embT[:], ept[:])
                for sub in range(XGRP // P):
                    esb = eopool.tile([P, ROW], bf16, tag="esb")
                    # emb node-major via PE transpose
                    ptr = epst.tile([P, HID], bf16, tag="t", space="PSUM")
                    nc.tensor.transpose(ptr[:], embT[:, sub * P:(sub + 1) * P],
                                        ident[0:HID, 0:HID])
                    nc.scalar.copy(esb[:, 0:HID], ptr[:])
                    nc.sync.dma_start(
                        etab_d[r0 + sub * P:r0 + (sub + 1) * P, :], esb[:])

        # ---------------- phase 2+3: GRU + attention + scatter ----------------
        NIDX = WALK * E_TILE
        with tc.tile_pool(name="g_idx", bufs=2) as ipool, \
             tc.tile_pool(name="g_gat", bufs=2) as gpool, \
             tc.tile_pool(name="g_rzn", bufs=2) as rznpool, \
             tc.tile_pool(name="g_h", bufs=3) as hspool, \
             tc.tile_pool(name="g_tmp", bufs=2) as tpool, \
             tc.tile_pool(name="g_pay", bufs=2) as paypool, \
             tc.tile_pool(name="g_oh", bufs=6) as ohpool, \
             tc.tile_pool(name="g_sp", bufs=2) as spool, \
             tc.tile_pool(name="g_ps", bufs=6, space="PSUM") as gpsum, \
             tc.tile_pool(name="p3_ps", bufs=2, space="PSUM") as p3psum:

            def wih_s(m):
                return wih_sb[:, m * P:(m + 1) * P]

            def whh_s(k, m):
                return whh_sb[:, k * G3 + m * P:k * G3 + (m + 1) * P]

            for t in range(n_tiles):
                idxt = ipool.tile([P, NIDX // 16], i16, tag="idx")
                nc.sync.dma_start(idxt[:], gidx_d[t])
                gat = gpool.tile([P, 1, NIDX], bf16, tag="gat")
                nc.gpsimd.dma_gather(gat[:], etab_d[:], idxt[:], NIDX, NIDX,
                                     ROW, transpose=True, single_packet=False)

                def x_s(s):
                    return gat[0:HID, 0, s * E_TILE:(s + 1) * E_TILE]

                h_cur = [None] * 4
                # ---- step 0 (h = 0): r gate irrelevant unless b_hh[n] != 0
                r0_sb = [None] * 4
                if has_bnhh:
                    for c in range(4):
                        p = gpsum.tile([P, E_TILE], f32, tag="g", space="PSUM")
                        nc.tensor.matmul(p[:], wih_s(c), x_s(0), start=True, stop=True)
                        r0 = rznpool.tile([P, E_TILE], f32, tag=f"rz{c}")
                        nc.scalar.activation(r0[:], p[:], AF.Sigmoid,
                                             bias=brz_sb[:, c:c + 1])
                        r0_sb[c] = r0
                z0_sb = [None] * 4
                for c in range(4):
                    p = gpsum.tile([P, E_TILE], f32, tag="g", space="PSUM")
                    nc.tensor.matmul(p[:], wih_s(4 + c), x_s(0), start=True, stop=True)
                    z0 = rznpool.tile([P, E_TILE], bf16, tag=f"rz{4 + c}")
                    nc.scalar.activation(z0[:], p[:], AF.Sigmoid,
                                         bias=brz_sb[:, 4 + c:5 + c])
                    z0_sb[c] = z0
                for c in range(4):
                    p = gpsum.tile([P, E_TILE], f32, tag="g", space="PSUM")
                    nc.tensor.matmul(p[:], wih_s(8 + c), x_s(0), start=True, stop=True)
                    n0 = rznpool.tile([P, E_TILE], bf16, tag=f"n{c}")
                    if has_bnhh:
                        rb = tpool.tile([P, E_TILE], f32, tag="rb")
                        nc.vector.tensor_scalar(rb[:], r0_sb[c][:],
                                                bnhh_sb[:, c:c + 1], None, OP.mult)
                        npre = tpool.tile([P, E_TILE], bf16, tag="npre")
                        nc.vector.tensor_tensor(npre[:], rb[:], p[:], OP.add)
                        nc.scalar.activation(n0[:], npre[:], AF.Tanh,
                                             bias=bnih_sb[:, c:c + 1])
                    else:
                        nc.scalar.activation(n0[:], p[:], AF.Tanh,
                                             bias=bnih_sb[:, c:c + 1])
                    zn = tpool.tile([P, E_TILE], bf16, tag="zn")
                    nc.vector.tensor_tensor(zn[:], z0_sb[c][:], n0[:], OP.mult)
                    h0 = hspool.tile([P, E_TILE], bf16, tag=f"h{c}")
                    nc.vector.tensor_tensor(h0[:], n0[:], zn[:], OP.subtract)
                    h_cur[c] = h0

                # ---- steps 1..3
                for s in range(1, WALK):
                    rz_sb = []
                    for m in range(8):
                        p = gpsum.tile([P, E_TILE], f32, tag="g", space="PSUM")
                        nc.tensor.matmul(p[:], wih_s(m), x_s(s),
                                         start=True, stop=False)
                        for k in range(4):
                            nc.tensor.matmul(p[:], whh_s(k, m), h_cur[k][:],
                                             start=False, stop=(k == 3))
                        rz = rznpool.tile([P, E_TILE], bf16, tag=f"rz{m}")
                        nc.scalar.activation(rz[:], p[:], AF.Sigmoid,
                                             bias=brz_sb[:, m:m + 1])
                        rz_sb.append(rz)
                    n_sb = []
                    for c in range(4):
                        pxn = gpsum.tile([P, E_TILE], f32, tag="g", space="PSUM")
                        nc.tensor.matmul(pxn[:], wih_s(8 + c), x_s(s),
                                         start=True, stop=True)
                        phn = gpsum.tile([P, E_TILE], f32, tag="g", space="PSUM")
                        for k in range(4):
                            nc.tensor.matmul(phn[:], whh_s(k, 8 + c), h_cur[k][:],
                                             start=(k == 0), stop=(k == 3))
                        rhn = tpool.tile([P, E_TILE], f32, tag="rhn")
                        if has_bnhh:
                            phb = tpool.tile([P, E_TILE], f32, tag="phb")
                            nc.vector.tensor_scalar(phb[:], phn[:],
                                                    bnhh_sb[:, c:c + 1], None, OP.add)
                            nc.vector.tensor_tensor(rhn[:], rz_sb[c][:], phb[:],
                                                    OP.mult)
                        else:
                            nc.vector.tensor_tensor(rhn[:], rz_sb[c][:], phn[:],
                                                    OP.mult)
                        npre = tpool.tile([P, E_TILE], bf16, tag="npre")
                        nc.vector.tensor_tensor(npre[:], rhn[:], pxn[:], OP.add)
                        nn = rznpool.tile([P, E_TILE], bf16, tag=f"n{c}")
                        nc.scalar.activation(nn[:], npre[:], AF.Tanh,
                                             bias=bnih_sb[:, c:c + 1])
                        n_sb.append(nn)
                    for c in range(4):
                        d = tpool.tile([P, E_TILE], bf16, tag="d")
                        nc.vector.tensor_tensor(d[:], h_cur[c][:], n_sb[c][:],
                                                OP.subtract)
                        zd = tpool.tile([P, E_TILE], bf16, tag="zd")
                        nc.vector.tensor_tensor(zd[:], rz_sb[4 + c][:], d[:], OP.mult)
                        hn = hspool.tile([P, E_TILE], bf16, tag=f"h{c}")
                        nc.vector.tensor_tensor(hn[:], n_sb[c][:], zd[:], OP.add)
                        h_cur[c] = hn

                # ---- attention payload per 128-edge chunk, then scatter
                pay = paypool.tile([P, KC, 136], bf16, tag="pay")
                for kc in range(KC):
                    k = t * KC + kc
                    if k not in chunk_js:
                        continue
                    pa = p3psum.tile([P, 136], f32, tag="p3", space="PSUM")
                    for kk in range(4):
                        nc.tensor.matmul(pa[:],
                                         h_cur[kk][:, kc * P:(kc + 1) * P],
                                         ba_sb[:, kk * 136:(kk + 1) * 136],
                                         start=(kk == 0), stop=(kk == 3))
                    asb = spool.tile([P, NH], f32, tag="asb")
                    nc.vector.tensor_scalar(asb[:], pa[:, 128:136], 0.01, None,
                                            OP.mult)
                    amx = spool.tile([P, NH], f32, tag="amx")
                    nc.vector.tensor_tensor(amx[:], pa[:, 128:136], asb[:], OP.max)
                    ea = spool.tile([P, NH], f32, tag="ea")
                    nc.scalar.activation(ea[:], amx[:], AF.Exp)
                    eae = spool.tile([P, NH, OUT_DIM], f32, tag="eae")
                    nc.vector.tensor_copy(
                        eae[:], ea[:, :, None].to_broadcast([P, NH, OUT_DIM]))
                    nc.vector.tensor_tensor(pay[:, kc, 0:128], pa[:, 0:128],
                                            eae[:].rearrange("p a b -> p (a b)"),
                                            OP.mult)
                    nc.scalar.copy(pay[:, kc, 128:136], ea[:])

                for j in tile_js[t]:
                    sc = p3psum.tile([P, 136], f32, tag="p3", space="PSUM")
                    ks = [kc for kc in range(KC)
                          if (t * KC + kc, j) in pair_of]
                    for i, kc in enumerate(ks):
                        pid = pair_of[(t * KC + kc, j)]
                        oh = ohpool.tile([P, P], bf16, tag="oh")
                        nc.sync.dma_start(oh[:], oneh_d[pid])
                        nc.tensor.matmul(sc[:], oh[:], pay[:, kc, :],
                                         start=(i == 0), stop=(i == len(ks) - 1),
                                         skip_group_check=True)
                    if first_t[j] == t:
                        nc.vector.tensor_copy(accs[:, j, :], sc[:])
                    else:
                        nc.vector.tensor_tensor(accs[:, j, :], accs[:, j, :],
                                                sc[:], OP.add)
                    if last_t[j] != t:
                        continue
                    # flush node-chunk j
                    scm = spool.tile([P, NH], f32, tag="scm")
                    nc.vector.tensor_scalar(scm[:], accs[:, j, 128:136], 1e-30,
                                            None, OP.max)
                    rc = spool.tile([P, NH], f32, tag="rc")
                    nc.vector.reciprocal(rc[:], scm[:])
                    rce = spool.tile([P, NH, OUT_DIM], f32, tag="rce")
                    nc.vector.tensor_copy(
                        rce[:], rc[:, :, None].to_broadcast([P, NH, OUT_DIM]))
                    wq = spool.tile([P, P], f32, tag="wq")
                    nc.vector.tensor_tensor(wq[:], accs[:, j, 0:128],
                                            rce[:].rearrange("p a b -> p (a b)"),
                                            OP.mult)
                    o16 = spool.tile([P, OUT_DIM], f32, tag="o16")
                    nc.vector.reduce_sum(
                        o16[:], wq[:].rearrange("p (h i) -> p i h", h=NH),
                        axis=mybir.AxisListType.X)
                    ob = spool.tile([P, OUT_DIM], f32, tag="ob")
                    nc.vector.tensor_tensor(ob[:], o16[:], bc_sb[:], OP.add)
                    nc.sync.dma_start(out_d[j * P:(j + 1) * P, :], ob[:])

    nc.compile()
    return nc


def kernel(**inputs):
    import os
    from concourse.bass_utils import run_bass_kernel_spmd

    num_nodes = int(inputs.pop("num_nodes", N_NODES))
    assert num_nodes == N_NODES
    plan, shared, percore = _host_prep(**inputs)
    nc = _build(plan)

    in_maps = []
    for c in range(N_CORES):
        m = dict(shared)
        m["gidx"] = np.ascontiguousarray(percore["gidx"][c])
        m["oneh"] = np.ascontiguousarray(percore["oneh"][c])
        in_maps.append(m)

    trace = bool(os.environ.get("KERNEL_TRACE"))
    res = run_bass_kernel_spmd(nc, in_maps, core_ids=list(range(N_CORES)),
                               trace=trace)
    global LAST_EXEC_NS, LAST_RESULTS
    LAST_EXEC_NS = getattr(res, "exec_time_ns", None)
    LAST_RESULTS = res

    full = np.empty((N_NODES, OUT_DIM), np.float32)
    for c in range(N_CORES):
        full[c * NPC:(c + 1) * NPC] = res.results[c]["out"][:NPC]
    # node chunks never flushed on device -> pure-bias rows
    for j in range(NODE_CHUNKS):
        if j not in plan["flushed"]:
            for c in range(N_CORES):
                lo = c * NPC + j * 128
                hi = min(c * NPC + min((j + 1) * 128, NPC), (c + 1) * NPC)
                if lo < hi:
                    full[lo:hi] = plan["bc_vec"][None, :]
    return full


# revision 15
# speedup vs baseline: 1.4240x; 1.1044x over previous
"""HGNN metapath GRU + edge-softmax message passing on 8 TRN2 NeuronCores.

Strategy (self-contained, full inputs in / full output out):
 - Edges are sharded by DESTINATION NODE RANGE: core c owns nodes
   [c*2500, (c+1)*2500) and every edge whose dst lands there (host sorts
   edges by dst).  All segment ops (softmax sum + message scatter) are then
   core-local: zero collectives.
 - The final two linear layers are folded through the segment-sum:
   out[n] = sum_h Q[n,h,:]/S[n,h] + bc,  where per-edge
   q[e,(h,i)] = exp(lrelu(a))[e,h] * (eft[e] @ BA)[.,(h,i)] is scattered
   with one-hot matmuls (one-hot matrices precomputed on host from indices).
 - Phase 1 consumes host-pre-transposed bf16 features (xT) so the node
   embedding table is built with N=512 feature-major matmuls + cheap PE
   transposes -- no fp32 transposes or casts.
 - GRU runs feature-major (gate dims on partitions, edges on the free dim);
   the transposing dma_gather returns table rows feature-major for free.
 - Attention + one-hot scatter are interleaved per edge-tile (no big hT
   buffer, no serial tail); per-node accumulators live in SBUF f32.
"""

import sys
import numpy as np

sys.path.insert(0, "/opt/trn_rl_repo")

import ml_dtypes  # noqa: E402

N_NODES = 20000
N_CORES = 8
NPC = N_NODES // N_CORES          # 2500 nodes per core
NODE_CHUNKS = (NPC + 127) // 128  # 20
WALK = 4
FEAT = 256
HID = 64
NH = 8
HR = NH * HID                     # 512
G3 = 3 * HR                       # 1536
OUT_DIM = 16
E_TILE = 512
KC = E_TILE // 128                # 128-edge scatter chunks per tile
ROW = 128                         # table row: emb(64) pad(64)
NP_PAD = ((N_NODES + 511) // 512) * 512  # 20480 padded node rows

bf = ml_dtypes.bfloat16


def _wrap_idx(v):
    """int array [n] -> wrapped int16 [128, n//16] layout for dma_gather."""
    n = v.shape[0]
    assert n % 16 == 0
    w = v.reshape(n // 16, 16).T.astype(np.int16)      # [16, n//16]
    return np.tile(w, (8, 1))                           # [128, n//16]


def _host_prep(x, W_mlp, b_mlp, W_ih, W_hh, b_ih, b_hh, attn, W_emb, b_emb,
               W_last, b_last, edge_metapath_indices):
    idx = np.asarray(edge_metapath_indices).astype(np.int64)
    dst = idx[:, -1]
    core = np.clip(dst // NPC, 0, N_CORES - 1)

    per_core_eids = []
    for c in range(N_CORES):
        sel = np.nonzero(core == c)[0]
        order = np.argsort(dst[sel], kind="stable")
        per_core_eids.append(sel[order])
    counts = [len(e) for e in per_core_eids]
    E_pad = max(E_TILE, ((max(counts) + E_TILE - 1) // E_TILE) * E_TILE)
    n_tiles = E_pad // E_TILE
    n_ech = E_pad // 128

    # per-core sorted/padded indices + local dst
    sidx = np.zeros((N_CORES, E_pad, WALK), np.int64)
    ldst = np.full((N_CORES, E_pad), -1000, np.int64)
    for c in range(N_CORES):
        e = per_core_eids[c]
        sidx[c, :len(e)] = idx[e]
        ldst[c, :len(e)] = dst[e] - c * NPC

    # gather indices: per tile, 4*E_TILE idxs (step-major)
    gidx = np.zeros((N_CORES, n_tiles, 128, (WALK * E_TILE) // 16), np.int16)
    for c in range(N_CORES):
        for t in range(n_tiles):
            v = sidx[c, t * E_TILE:(t + 1) * E_TILE, :].T.reshape(-1)
            gidx[c, t] = _wrap_idx(v)

    # shared scatter schedule: union over cores of node-chunks touched per
    # 128-edge chunk k
    pairs = []            # list of (k, j)
    pair_of = {}
    for k in range(n_ech):
        js = set()
        for c in range(N_CORES):
            d = ldst[c, k * 128:(k + 1) * 128]
            js |= set((d[d >= 0] // 128).tolist())
        if js:
            for j in range(min(js), max(js) + 1):
                pair_of[(k, j)] = len(pairs)
                pairs.append((k, j))
    first_k, last_k = {}, {}
    for (k, j) in pairs:
        first_k.setdefault(j, k)
        last_k[j] = k
    n_pairs = len(pairs)

    oneh = np.zeros((N_CORES, max(n_pairs, 1), 128, 128), bf)
    m_ids = np.arange(128)
    for c in range(N_CORES):
        for p, (k, j) in enumerate(pairs):
            d = ldst[c, k * 128:(k + 1) * 128]
            oneh[c, p] = (d[:, None] == (j * 128 + m_ids)[None, :]).astype(bf)

    # weights
    Wc = (np.asarray(W_last, np.float32) @ np.asarray(W_emb, np.float32))  # [16, 512]
    BA = np.zeros((HR, 136), np.float32)
    attn = np.asarray(attn, np.float32)
    for h in range(NH):
        BA[h * HID:(h + 1) * HID, h * OUT_DIM:(h + 1) * OUT_DIM] = \
            Wc[:, h * HID:(h + 1) * HID].T
        BA[h * HID:(h + 1) * HID, 128 + h] = attn[0, h, :]
    ba_p = BA.reshape(4, 128, 136).transpose(1, 0, 2).reshape(128, 4 * 136).astype(bf)

    W_hhT = np.asarray(W_hh, np.float32).T                       # [512, 1536]
    whh_p = W_hhT.reshape(4, 128, G3).transpose(1, 0, 2).reshape(128, 4 * G3).astype(bf)
    wih_p = np.asarray(W_ih, np.float32).T.astype(bf)            # [64, 1536]

    b_ih = np.asarray(b_ih, np.float32)
    b_hh = np.asarray(b_hh, np.float32)
    brz = (b_ih + b_hh)[:2 * HR].reshape(8, 128).T.copy()        # [128, 8]
    bnih = b_ih[2 * HR:].reshape(4, 128).T.copy()                # [128, 4]
    bnhh = b_hh[2 * HR:].reshape(4, 128).T.copy()                # [128, 4]
    has_bnhh = bool(np.any(bnhh != 0.0))

    b_mlp = np.asarray(b_mlp, np.float32)
    has_bmlp = bool(np.any(b_mlp != 0.0))
    bmlp_col = b_mlp.reshape(HID, 1).astype(np.float32)          # [64, 1]

    bc_vec = (np.asarray(b_emb, np.float32) @ np.asarray(W_last, np.float32).T
              + np.asarray(b_last, np.float32))                  # [16]
    bc_t = np.tile(bc_vec[None, :], (128, 1)).astype(np.float32)

    # host-pre-transposed bf16 node features [256, NP_PAD]
    x_pad = np.zeros((NP_PAD, FEAT), np.float32)
    x_pad[:N_NODES] = np.asarray(x, np.float32)
    xT = np.ascontiguousarray(x_pad.T.astype(bf))                # [256, NP_PAD]

    # W_mlp.T packed by k-chunk: [128, 2*HID] bf16
    wmlp_p = np.asarray(W_mlp, np.float32).T.astype(bf)          # [256, 64]
    wmlp_pk = wmlp_p.reshape(2, 128, HID).transpose(1, 0, 2).reshape(128, 2 * HID)

    plan = dict(E_pad=E_pad, n_tiles=n_tiles, n_ech=n_ech, pairs=pairs,
                pair_of=pair_of, first_k=first_k, last_k=last_k,
                n_pairs=n_pairs, has_bnhh=has_bnhh, has_bmlp=has_bmlp,
                flushed=set(last_k.keys()), bc_vec=bc_vec)
    shared = dict(xt=xT, wmlp=np.ascontiguousarray(wmlp_pk), wih=wih_p,
                  whh=whh_p, ba=ba_p,
                  brz=brz, bnih=bnih, bnhh=bnhh, bmlp=bmlp_col, bc=bc_t)
    percore = dict(gidx=gidx, oneh=oneh)
    return plan, shared, percore


def _build(plan):
    from contextlib import ExitStack
    import concourse.bass as bass
    import concourse.tile as tile
    from concourse import bacc, mybir

    f32 = mybir.dt.float32
    bf16 = mybir.dt.bfloat16
    i16 = mybir.dt.int16
    AF = mybir.ActivationFunctionType
    OP = mybir.AluOpType
    P = 128

    E_pad, n_tiles, n_ech = plan["E_pad"], plan["n_tiles"], plan["n_ech"]
    pairs, pair_of = plan["pairs"], plan["pair_of"]
    first_k, last_k = plan["first_k"], plan["last_k"]
    has_bnhh, has_bmlp = plan["has_bnhh"], plan["has_bmlp"]

    nc = bacc.Bacc("TRN2", target_bir_lowering=False, debug=False)

    xt_d = nc.dram_tensor("xt", [FEAT, NP_PAD], bf16, kind="ExternalInput")
    wmlp_d = nc.dram_tensor("wmlp", [P, 2 * HID], bf16, kind="ExternalInput")
    wih_d = nc.dram_tensor("wih", [HID, G3], bf16, kind="ExternalInput")
    whh_d = nc.dram_tensor("whh", [P, 4 * G3], bf16, kind="ExternalInput")
    ba_d = nc.dram_tensor("ba", [P, 4 * 136], bf16, kind="ExternalInput")
    brz_d = nc.dram_tensor("brz", [P, 8], f32, kind="ExternalInput")
    bnih_d = nc.dram_tensor("bnih", [P, 4], f32, kind="ExternalInput")
    bnhh_d = nc.dram_tensor("bnhh", [P, 4], f32, kind="ExternalInput")
    bmlp_d = nc.dram_tensor("bmlp", [HID, 1], f32, kind="ExternalInput")
    bc_d = nc.dram_tensor("bc", [P, OUT_DIM], f32, kind="ExternalInput")
    gidx_d = nc.dram_tensor("gidx", [n_tiles, P, (WALK * E_TILE) // 16], i16,
                            kind="ExternalInput")
    oneh_d = nc.dram_tensor("oneh", [max(plan["n_pairs"], 1), P, P], bf16,
                            kind="ExternalInput")
    out_d = nc.dram_tensor("out", [NODE_CHUNKS * P, OUT_DIM], f32,
                           kind="ExternalOutput")
    etab_d = nc.dram_tensor("etab", [NP_PAD, ROW], bf16, kind="Internal")

    # per-tile scatter schedule
    # batched scatter schedule: payload/exp/scatter run once per BG tiles
    BG = 4
    KB = BG * KC                        # 128-edge chunks per batch
    n_b = (n_tiles + BG - 1) // BG
    chunk_js = {}                       # k -> [j...]
    for (k, j) in pairs:
        chunk_js.setdefault(k, []).append(j)
    batch_js = []                       # b -> sorted js touched in batch b
    for b in range(n_b):
        js = set()
        for k in range(b * KB, min((b + 1) * KB, n_ech)):
            js |= set(chunk_js.get(k, []))
        batch_js.append(sorted(js))
    first_b = {j: first_k[j] // KB for j in first_k}
    last_b = {j: last_k[j] // KB for j in last_k}

    from concourse.masks import make_identity

    with tile.TileContext(nc) as tc, ExitStack() as ctx:
        wpool = ctx.enter_context(tc.tile_pool(name="w", bufs=1))
        wih_sb = wpool.tile([HID, G3], bf16, tag="wih")
        nc.sync.dma_start(wih_sb[:], wih_d[:])
        whh_sb = wpool.tile([P, 4 * G3], bf16, tag="whh")
        nc.sync.dma_start(whh_sb[:], whh_d[:])
        ba_sb = wpool.tile([P, 4 * 136], bf16, tag="ba")
        nc.sync.dma_start(ba_sb[:], ba_d[:])
        brz_sb = wpool.tile([P, 8], f32, tag="brz")
        nc.sync.dma_start(brz_sb[:], brz_d[:])
        bnih_sb = wpool.tile([P, 4], f32, tag="bnih")
        nc.sync.dma_start(bnih_sb[:], bnih_d[:])
        bnhh_sb = wpool.tile([P, 4], f32, tag="bnhh")
        nc.sync.dma_start(bnhh_sb[:], bnhh_d[:])
        bmlp_sb = wpool.tile([HID, 1], f32, tag="bmlp")
        nc.sync.dma_start(bmlp_sb[:], bmlp_d[:])
        bc_sb = wpool.tile([P, OUT_DIM], f32, tag="bc")
        nc.sync.dma_start(bc_sb[:], bc_d[:])
        wm_sb = wpool.tile([P, 2 * HID], bf16, tag="wm")
        nc.sync.dma_start(wm_sb[:], wmlp_d[:])
        ident = wpool.tile([P, P], bf16, tag="ident")
        make_identity(nc, ident[:])
        # per-node-chunk SBUF accumulators (exp-weighted features + exp sums)
        accs = wpool.tile([P, NODE_CHUNKS, 136], f32, tag="accs")
        nc.vector.memset(accs[:], 0)

        # ---------------- phase 1: node embedding table ----------------
        # etab columns 64:128 hold garbage (never read: gather consumers
        # slice [0:HID]).
        XGRP = 512
        with tc.tile_pool(name="e_x", bufs=3) as expool, \
             tc.tile_pool(name="e_et", bufs=3) as etpool, \
             tc.tile_pool(name="e_o", bufs=6) as eopool, \
             tc.tile_pool(name="e_pe", bufs=2, space="PSUM") as epse, \
             tc.tile_pool(name="e_pt", bufs=4, space="PSUM") as epst:
            for g in range(NP_PAD // XGRP):  # 40 groups
                r0 = g * XGRP
                xa = expool.tile([P, XGRP], bf16, tag="xa")
                nc.sync.dma_start(xa[:], xt_d[0:P, r0:r0 + XGRP])
                xb = expool.tile([P, XGRP], bf16, tag="xb")
                nc.scalar.dma_start(xb[:], xt_d[P:2 * P, r0:r0 + XGRP])
                # embT [64, XGRP] = W_mlp @ x.T  (feature-major)
                ept = epse.tile([HID, XGRP], f32, tag="e", space="PSUM")
                nc.tensor.matmul(ept[:], wm_sb[:, 0:HID], xa[:],
                                 start=True, stop=False)
                nc.tensor.matmul(ept[:], wm_sb[:, HID:2 * HID], xb[:],
                                 start=False, stop=True)
                embT = etpool.tile([HID, XGRP], bf16, tag="embT")
                if has_bmlp:
                    nc.vector.tensor_scalar(embT[:], ept[:], bmlp_sb[:],
                                            None, OP.add)
                else:
                    nc.scalar.copy(embT[:], ept[:])
                for sub in range(XGRP // P):
                    esb = eopool.tile([P, ROW], bf16, tag="esb")
                    # emb node-major via PE transpose
                    ptr = epst.tile([P, HID], bf16, tag="t", space="PSUM")
                    nc.tensor.transpose(ptr[:], embT[:, sub * P:(sub + 1) * P],
                                        ident[0:HID, 0:HID])
                    nc.scalar.copy(esb[:, 0:HID], ptr[:])
                    nc.gpsimd.dma_start(
                        etab_d[r0 + sub * P:r0 + (sub + 1) * P, :], esb[:])

        # ---------------- phase 2+3: GRU + attention + scatter ----------------
        NIDX = WALK * E_TILE
        with tc.tile_pool(name="g_idx", bufs=2) as ipool, \
             tc.tile_pool(name="g_gat", bufs=2) as gpool, \
             tc.tile_pool(name="g_rzn", bufs=2) as rznpool, \
             tc.tile_pool(name="g_h", bufs=3) as hspool, \
             tc.tile_pool(name="g_tmp", bufs=2) as tpool, \
             tc.tile_pool(name="g_pay", bufs=2) as paypool, \
             tc.tile_pool(name="g_oh", bufs=6) as ohpool, \
             tc.tile_pool(name="g_sp", bufs=2) as spool, \
             tc.tile_pool(name="g_ps", bufs=6, space="PSUM") as gpsum, \
             tc.tile_pool(name="p3_ps", bufs=2, space="PSUM") as p3psum:

            def wih_s(m):
                return wih_sb[:, m * P:(m + 1) * P]

            def whh_s(k, m):
                return whh_sb[:, k * G3 + m * P:k * G3 + (m + 1) * P]

            for t in range(n_tiles):
                idxt = ipool.tile([P, NIDX // 16], i16, tag="idx")
                nc.sync.dma_start(idxt[:], gidx_d[t])
                gat = gpool.tile([P, 1, NIDX], bf16, tag="gat")
                nc.gpsimd.dma_gather(gat[:], etab_d[:], idxt[:], NIDX, NIDX,
                                     ROW, transpose=True, single_packet=False)

                def x_s(s):
                    return gat[0:HID, 0, s * E_TILE:(s + 1) * E_TILE]

                h_cur = [None] * 4
                # ---- step 0 (h = 0): r gate irrelevant unless b_hh[n] != 0
                r0_sb = [None] * 4
                if has_bnhh:
                    for c in range(4):
                        p = gpsum.tile([P, E_TILE], f32, tag="g", space="PSUM")
                        nc.tensor.matmul(p[:], wih_s(c), x_s(0), start=True, stop=True)
                        r0 = rznpool.tile([P, E_TILE], f32, tag=f"rz{c}")
                        nc.scalar.activation(r0[:], p[:], AF.Sigmoid,
                                             bias=brz_sb[:, c:c + 1])
                        r0_sb[c] = r0
                z0_sb = [None] * 4
                for c in range(4):
                    p = gpsum.tile([P, E_TILE], f32, tag="g", space="PSUM")
                    nc.tensor.matmul(p[:], wih_s(4 + c), x_s(0), start=True, stop=True)
                    z0 = rznpool.tile([P, E_TILE], bf16, tag=f"rz{4 + c}")
                    nc.scalar.activation(z0[:], p[:], AF.Sigmoid,
                                         bias=brz_sb[:, 4 + c:5 + c])
                    z0_sb[c] = z0
                for c in range(4):
                    p = gpsum.tile([P, E_TILE], f32, tag="g", space="PSUM")
                    nc.tensor.matmul(p[:], wih_s(8 + c), x_s(0), start=True, stop=True)
                    n0 = rznpool.tile([P, E_TILE], bf16, tag=f"n{c}")
                    if has_bnhh:
                        rb = tpool.tile([P, E_TILE], f32, tag="rb")
                        nc.vector.tensor_scalar(rb[:], r0_sb[c][:],
                                                bnhh_sb[:, c:c + 1], None, OP.mult)
                        npre = tpool.tile([P, E_TILE], bf16, tag="npre")
                        nc.vector.tensor_tensor(npre[:], rb[:], p[:], OP.add)
                        nc.scalar.activation(n0[:], npre[:], AF.Tanh,
                                             bias=bnih_sb[:, c:c + 1])
                    else:
                        nc.scalar.activation(n0[:], p[:], AF.Tanh,
                                             bias=bnih_sb[:, c:c + 1])
                    zn = tpool.tile([P, E_TILE], bf16, tag="zn")
                    nc.vector.tensor_tensor(zn[:], z0_sb[c][:], n0[:], OP.mult)
                    h0 = hspool.tile([P, E_TILE], bf16, tag=f"h{c}")
                    nc.vector.tensor_tensor(h0[:], n0[:], zn[:], OP.subtract)
                    h_cur[c] = h0

                # ---- steps 1..3
                for s in range(1, WALK):
                    rz_sb = []
                    for m in range(8):
                        p = gpsum.tile([P, E_TILE], f32, tag="g", space="PSUM")
                        nc.tensor.matmul(p[:], wih_s(m), x_s(s),
                                         start=True, stop=False)
                        for k in range(4):
                            nc.tensor.matmul(p[:], whh_s(k, m), h_cur[k][:],
                                             start=False, stop=(k == 3))
                        rz = rznpool.tile([P, E_TILE], bf16, tag=f"rz{m}")
                        nc.scalar.activation(rz[:], p[:], AF.Sigmoid,
                                             bias=brz_sb[:, m:m + 1])
                        rz_sb.append(rz)
                    n_sb = []
                    for c in range(4):
                        pxn = gpsum.tile([P, E_TILE], f32, tag="g", space="PSUM")
                        nc.tensor.matmul(pxn[:], wih_s(8 + c), x_s(s),
                                         start=True, stop=True)
                        phn = gpsum.tile([P, E_TILE], f32, tag="g", space="PSUM")
                        for k in range(4):
                            nc.tensor.matmul(phn[:], whh_s(k, 8 + c), h_cur[k][:],
                                             start=(k == 0), stop=(k == 3))
                        rhn = tpool.tile([P, E_TILE], f32, tag="rhn")
                        if has_bnhh:
                            phb = tpool.tile([P, E_TILE], f32, tag="phb")
                            nc.vector.tensor_scalar(phb[:], phn[:],
                                                    bnhh_sb[:, c:c + 1], None, OP.add)
                            nc.vector.tensor_tensor(rhn[:], rz_sb[c][:], phb[:],
                                                    OP.mult)
                        else:
                            nc.vector.tensor_tensor(rhn[:], rz_sb[c][:], phn[:],
                                                    OP.mult)
                        npre = tpool.tile([P, E_TILE], bf16, tag="npre")
                        nc.vector.tensor_tensor(npre[:], rhn[:], pxn[:], OP.add)
                        nn = rznpool.tile([P, E_TILE], bf16, tag=f"n{c}")
                        nc.scalar.activation(nn[:], npre[:], AF.Tanh,
                                             bias=bnih_sb[:, c:c + 1])
                        n_sb.append(nn)
                    for c in range(4):
                        d = tpool.tile([P, E_TILE], bf16, tag="d")
                        nc.vector.tensor_tensor(d[:], h_cur[c][:], n_sb[c][:],
                                                OP.subtract)
                        zd = tpool.tile([P, E_TILE], bf16, tag="zd")
                        nc.vector.tensor_tensor(zd[:], rz_sb[4 + c][:], d[:], OP.mult)
                        hn = hspool.tile([P, E_TILE], bf16, tag=f"h{c}")
                        nc.vector.tensor_tensor(hn[:], n_sb[c][:], zd[:], OP.add)
                        h_cur[c] = hn

                # ---- attention logits+payload matmuls per 128-edge chunk
                if t % BG == 0:
                    pstor = paypool.tile([P, KB, 136], f32, tag="pstor")
                for kc in range(KC):
                    k = t * KC + kc
                    if k not in chunk_js:
                        continue
                    pa = p3psum.tile([P, 136], f32, tag="p3", space="PSUM")
                    for kk in range(4):
                        nc.tensor.matmul(pa[:],
                                         h_cur[kk][:, kc * P:(kc + 1) * P],
                                         ba_sb[:, kk * 136:(kk + 1) * 136],
                                         start=(kk == 0), stop=(kk == 3))
                    nc.vector.tensor_copy(pstor[:, (t % BG) * KC + kc, :], pa[:])

                # ---- every BG tiles: exp + weighting + scatter + flush
                if t % BG != BG - 1 and t != n_tiles - 1:
                    continue
                b = t // BG
                nb = (t % BG) + 1            # tiles in this batch
                nk = nb * KC                 # 128-edge chunks in this batch
                asb = spool.tile([P, KB, NH], f32, tag="asb")
                nc.vector.tensor_scalar(asb[:, 0:nk, :], pstor[:, 0:nk, 128:136],
                                        0.01, None, OP.mult)
                amx = spool.tile([P, KB * NH], f32, tag="amx")
                am3 = amx[:].rearrange("p (k a) -> p k a", a=NH)
                nc.vector.tensor_tensor(am3[:, 0:nk, :], pstor[:, 0:nk, 128:136],
                                        asb[:, 0:nk, :], OP.max)
                ea = spool.tile([P, KB * NH], f32, tag="ea")
                nc.scalar.activation(ea[:, 0:nk * NH], amx[:, 0:nk * NH], AF.Exp)
                ea3 = ea[:].rearrange("p (k a) -> p k a", a=NH)
                eae = spool.tile([P, KB, NH, OUT_DIM], f32, tag="eae")
                nc.vector.tensor_copy(
                    eae[:, 0:nk], ea3[:, 0:nk, :, None]
                    .to_broadcast([P, nk, NH, OUT_DIM]))
                pay = paypool.tile([P, KB, 136], bf16, tag="pay")
                nc.vector.tensor_tensor(
                    pay[:, 0:nk, 0:128], pstor[:, 0:nk, 0:128],
                    eae[:, 0:nk].rearrange("p k a b -> p k (a b)"), OP.mult)
                nc.vector.tensor_copy(pay[:, 0:nk, 128:136], ea3[:, 0:nk, :])

                for j in batch_js[b]:
                    sc = p3psum.tile([P, 136], f32, tag="p3", space="PSUM")
                    ks = [k for k in range(b * KB, b * KB + nk)
                          if (k, j) in pair_of]
                    for i, k in enumerate(ks):
                        pid = pair_of[(k, j)]
                        oh = ohpool.tile([P, P], bf16, tag="oh")
                        nc.sync.dma_start(oh[:], oneh_d[pid])
                        nc.tensor.matmul(sc[:], oh[:], pay[:, k - b * KB, :],
                                         start=(i == 0), stop=(i == len(ks) - 1),
                                         skip_group_check=True)
                    if first_b[j] == b:
                        nc.vector.tensor_copy(accs[:, j, :], sc[:])
                    else:
                        nc.vector.tensor_tensor(accs[:, j, :], accs[:, j, :],
                                                sc[:], OP.add)
                    if last_b[j] != b:
                        continue
                    # flush node-chunk j
                    scm = spool.tile([P, NH], f32, tag="scm")
                    nc.vector.tensor_scalar(scm[:], accs[:, j, 128:136], 1e-30,
                                            None, OP.max)
                    rc = spool.tile([P, NH], f32, tag="rc")
                    nc.vector.reciprocal(rc[:], scm[:])
                    rce = spool.tile([P, NH, OUT_DIM], f32, tag="rce")
                    nc.vector.tensor_copy(
                        rce[:], rc[:, :, None].to_broadcast([P, NH, OUT_DIM]))
                    wq = spool.tile([P, P], f32, tag="wq")
                    nc.vector.tensor_tensor(wq[:], accs[:, j, 0:128],
                                            rce[:].rearrange("p a b -> p (a b)"),
                                            OP.mult)
                    o16 = spool.tile([P, OUT_DIM], f32, tag="o16")
                    nc.vector.reduce_sum(
                        o16[:], wq[:].rearrange("p (h i) -> p i h", h=NH),
                        axis=mybir.AxisListType.X)
                    ob = spool.tile([P, OUT_DIM], f32, tag="ob")
                    nc.vector.tensor_tensor(ob[:], o16[:], bc_sb[:], OP.add)
                    nc.sync.dma_start(out_d[j * P:(j + 1) * P, :], ob[:])

    nc.compile()
    return nc


def kernel(**inputs):
    import os
    from concourse.bass_utils import run_bass_kernel_spmd

    num_nodes = int(inputs.pop("num_nodes", N_NODES))
    assert num_nodes == N_NODES
    plan, shared, percore = _host_prep(**inputs)
    nc = _build(plan)

    in_maps = []
    for c in range(N_CORES):
        m = dict(shared)
        m["gidx"] = np.ascontiguousarray(percore["gidx"][c])
        m["oneh"] = np.ascontiguousarray(percore["oneh"][c])
        in_maps.append(m)

    trace = bool(os.environ.get("KERNEL_TRACE"))
    res = run_bass_kernel_spmd(nc, in_maps, core_ids=list(range(N_CORES)),
                               trace=trace)
    global LAST_EXEC_NS, LAST_RESULTS
    LAST_EXEC_NS = getattr(res, "exec_time_ns", None)
    LAST_RESULTS = res

    full = np.empty((N_NODES, OUT_DIM), np.float32)
    for c in range(N_CORES):
        full[c * NPC:(c + 1) * NPC] = res.results[c]["out"][:NPC]
    # node chunks never flushed on device -> pure-bias rows
    for j in range(NODE_CHUNKS):
        if j not in plan["flushed"]:
            for c in range(N_CORES):
                lo = c * NPC + j * 128
                hi = min(c * NPC + min((j + 1) * 128, NPC), (c + 1) * NPC)
                if lo < hi:
                    full[lo:hi] = plan["bc_vec"][None, :]
    return full
